# revision 3
# baseline (speedup 1.0000x reference)
"""CGConv message-passing kernel for 8 Trainium2 NeuronCores.

Strategy (self-contained; shapes hardcoded for the nn_CGConv problem):
 - Sort edges by destination node (col); shard edges into 8 buckets by
   col-range so every core owns a disjoint slice of output nodes (no
   collective needed).
 - Host precomputes the per-node projections xg1 = x@gw1.T, xg2 = x@gw2.T
   (and msg twins) and ships per-edge pre-added tables
   G = -(xg1[row]+xg2[col]) and M = xm1[row]+xm2[col] as channel-major
   bf16 so the device streams them with contiguous DMA.  The gate side is
   negated so exp(gate_pre) == exp(-g) without an extra scale.
 - Device, per 1024-edge pair of tiles: inject G/M into PSUM via an
   identity matmul, accumulate the edge_attr projection (bias folded in
   via a ones-row), then t1=exp(-g), t2=exp(c), sp=ln(1+t2) on the ACT
   engine (natural_log_exp table pinned) and m = sp/(1+t1) on DVE.
 - Messages are PE-transposed to [edge, ch] and segment-summed by
   destination via host-built one-hot matmuls into a 64-node window per
   512-edge tile; window sums stream back to DRAM in group batches.
 - DMAs are issued per 5-pair group (5120 edges) to keep the Sync
   engine's descriptor-issue cost off the critical path.
 - The host merges windows, adds the residual, and assembles the full
   [25000, 128] output.
"""

import numpy as np
import ml_dtypes

BF16 = ml_dtypes.bfloat16

N_NODES = 25000
N_EDGES = 400000
C = 128
EC = 64
N_CORES = 8
SHARD = 3125          # nodes per core
SHARD_PAD = 3200      # padded (multiple of 128)
TILE = 512            # edges per tile (one segment window each)
PAIR = 1024           # edges per compute step
GROUP_PAIRS = 5       # pairs per DMA group


def _prep(x, edge_index, edge_attr, gate_w, gate_b, msg_w, msg_b):
    row = np.asarray(edge_index[0], dtype=np.int64)
    col = np.asarray(edge_index[1], dtype=np.int64)
    x = np.asarray(x, dtype=np.float32)
    edge_attr = np.asarray(edge_attr, dtype=np.float32)
    gw = np.asarray(gate_w, np.float32)
    mw = np.asarray(msg_w, np.float32)
    gb = np.asarray(gate_b, np.float32)
    mb = np.asarray(msg_b, np.float32)

    order = np.argsort(col, kind="stable")
    row_s, col_s = row[order], col[order]
    attr_s = edge_attr[order]

    shard_of = col_s // SHARD
    starts = np.searchsorted(shard_of, np.arange(N_CORES))
    ends = np.searchsorted(shard_of, np.arange(N_CORES), side="right")
    sizes = ends - starts
    e_pad = int(-(-sizes.max() // PAIR) * PAIR)
    n_pairs = e_pad // PAIR
    n_sup = e_pad // TILE

    # per-node projections (f32 on host; quantized to bf16 after pre-add)
    xg1 = x @ gw[:, 0:C].T
    xg2 = x @ gw[:, C:2 * C].T
    xm1 = x @ mw[:, 0:C].T
    xm2 = x @ mw[:, C:2 * C].T

    # segment-window width: covers the max col-span of any 512-edge tile
    max_span = 0
    for i in range(N_CORES):
        c_l = col_s[starts[i]:ends[i]] - i * SHARD
        for t0 in range(0, len(c_l), TILE):
            seg = c_l[t0:t0 + TILE]
            max_span = max(max_span, int(seg[-1]) - int(seg[0]))
    Wseg = int(-(-(max_span + 3) // 32) * 32)  # pad-edge slack + round to 32
    Wseg = max(64, min(Wseg, 192))

    in_maps = []
    merge_info = []
    for i in range(N_CORES):
        sl = slice(starts[i], ends[i])
        r_i = row_s[sl]
        c_loc = (col_s[sl] - i * SHARD).astype(np.int64)
        a_i = attr_s[sl]
        n_i = sizes[i]
        pad = e_pad - n_i

        # pad col: must be >= SHARD (dropped at merge) and within W of its
        # tile's base. Real cols end <= 3124; pads go to a dedicated node id.
        if pad:
            last_real = int(c_loc[-1]) if n_i else 0
            pad_col = max(SHARD, last_real + 1)
            assert pad_col < SHARD_PAD
            r_i = np.concatenate([r_i, np.zeros(pad, np.int64)])
            c_loc = np.concatenate([c_loc, np.full(pad, pad_col, np.int64)])
            a_i = np.concatenate([a_i, np.zeros((pad, EC), np.float32)])

        bases = []
        for t in range(n_sup):
            seg = c_loc[t * TILE:(t + 1) * TILE]
            b = int(seg[0])
            assert int(seg[-1]) - b < Wseg, (
                f"core {i} tile {t}: col span {int(seg[-1]) - b} >= W={Wseg}")
            bases.append(b)
        bases_arr = np.repeat(np.asarray(bases, np.int64), TILE)

        c_glob = np.minimum(c_loc + i * SHARD, N_NODES - 1)
        # pre-added row+col projections, channel-major; gate side negated so
        # the device's exp(gate_psum) directly yields exp(-g)
        G = np.ascontiguousarray(-(xg1[r_i] + xg2[c_glob]).T).astype(BF16)
        M = np.ascontiguousarray((xm1[r_i] + xm2[c_glob]).T).astype(BF16)

        attrT = np.empty((EC + 1, e_pad), dtype=BF16)
        attrT[:EC] = a_i.T
        attrT[EC] = 1.0

        # one-hot segment matrices: B[e, w] = (col[e] - base == w), stored
        # [128 (edge-in-chunk), n_chunks * W] so chunk c sits at cols c*W
        colsub = (c_loc - bases_arr).astype(np.int64)
        assert colsub.min() >= 0 and colsub.max() < Wseg
        bmat = np.zeros((e_pad, Wseg), dtype=BF16)
        bmat[np.arange(e_pad), colsub] = 1.0
        bmat = np.ascontiguousarray(
            bmat.reshape(e_pad // 128, 128, Wseg).transpose(1, 0, 2)
        ).reshape(128, -1)

        in_maps.append({"preG": G, "preM": M, "attrT": attrT, "bmat": bmat})
        merge_info.append((bases, n_i))

    # shared (replicated) weight tables; gate side negated (bias folded)
    w3g = np.empty((EC + 1, C), dtype=BF16)
    w3g[:EC] = -gw[:, 2 * C:].T
    w3g[EC] = -gb
    w3m = np.empty((EC + 1, C), dtype=BF16)
    w3m[:EC] = mw[:, 2 * C:].T
    w3m[EC] = mb
    ident = np.eye(128, dtype=BF16)
    shared = {"w3g": w3g, "w3m": w3m, "ident": ident}
    for m in in_maps:
        m.update(shared)

    meta = {"e_pad": e_pad, "n_pairs": n_pairs, "n_sup": n_sup, "w_seg": Wseg}
    return in_maps, meta, merge_info


def _build(meta):
    import concourse.bacc as bacc
    import concourse.mybir as mybir
    from concourse import tile
    from concourse.alu_op_type import AluOpType

    e_pad, n_pairs, W = meta["e_pad"], meta["n_pairs"], meta["w_seg"]
    n_groups = -(-n_pairs // GROUP_PAIRS)
    assert n_pairs % GROUP_PAIRS == 0, "pad e_pad to a group multiple"
    GE = GROUP_PAIRS * PAIR              # edges per group
    bf = mybir.dt.bfloat16
    f32 = mybir.dt.float32
    AF = mybir.ActivationFunctionType

    nc = bacc.Bacc(None, target_bir_lowering=False, debug=False)

    preG_d = nc.declare_dram_parameter("preG", [C, e_pad], bf, isOutput=False)
    preM_d = nc.declare_dram_parameter("preM", [C, e_pad], bf, isOutput=False)
    attr_d = nc.declare_dram_parameter("attrT", [EC + 1, e_pad], bf, isOutput=False)
    bmat_d = nc.declare_dram_parameter("bmat", [128, (e_pad // 128) * W], bf, isOutput=False)
    ident_d = nc.declare_dram_parameter("ident", [128, 128], bf, isOutput=False)
    w3g_d = nc.declare_dram_parameter("w3g", [EC + 1, C], bf, isOutput=False)
    w3m_d = nc.declare_dram_parameter("w3m", [EC + 1, C], bf, isOutput=False)
    wsums_d = nc.declare_dram_parameter("wsums", [C, (e_pad // TILE) * W], f32, isOutput=True)

    with tile.TileContext(nc) as tc:
        with (
            tc.tile_pool(name="const", bufs=1) as cpool,
            tc.tile_pool(name="grp", bufs=2) as gpool,
            tc.tile_pool(name="wout", bufs=2) as wpool,
            tc.tile_pool(name="elem", bufs=3) as epool,
            tc.tile_pool(name="mps", bufs=2, space="PSUM") as main_pool,
            tc.tile_pool(name="tps", bufs=2, space="PSUM") as t_pool,
            tc.tile_pool(name="sps", bufs=2, space="PSUM") as s_pool,
        ):
            ident = cpool.tile([128, 128], bf, tag="ident")
            nc.sync.dma_start(ident[:], ident_d[:])
            w3g = cpool.tile([EC + 1, C], bf, tag="w3g")
            nc.sync.dma_start(w3g[:], w3g_d[:])
            w3m = cpool.tile([EC + 1, C], bf, tag="w3m")
            nc.sync.dma_start(w3m[:], w3m_d[:])

            # ~8us of dense back-to-back matmuls: one continuous burst longer
            # than the HAM activity window so the PE clock unthrottles to
            # 2.4GHz before the real stream.
            warm_in = cpool.tile([128, TILE], bf, tag="warm")
            nc.gpsimd.memset(warm_in[:], 0.0)
            warm_ps = s_pool.tile([C, W], f32, tag="seg", name="warm_ps")
            for _ in range(90):
                nc.tensor.matmul(warm_ps[:], warm_in[:, 0:128], warm_in[:, 0:W],
                                 start=True, stop=True, skip_group_check=True)

            for g in range(n_groups):
                gsl = slice(g * GE, (g + 1) * GE)
                preG_t = gpool.tile([C, GE], bf, tag="pg")
                nc.sync.dma_start(preG_t[:], preG_d[:, gsl])
                preM_t = gpool.tile([C, GE], bf, tag="pm")
                nc.sync.dma_start(preM_t[:], preM_d[:, gsl])
                attr_t = gpool.tile([EC + 1, GE], bf, tag="at")
                nc.sync.dma_start(attr_t[:], attr_d[:, gsl])
                b_t = gpool.tile([128, GROUP_PAIRS * 8 * W], bf, tag="bt")
                nc.sync.dma_start(b_t[:], bmat_d[:, g * GROUP_PAIRS * 8 * W:
                                                 (g + 1) * GROUP_PAIRS * 8 * W])
                wstage = wpool.tile([C, GROUP_PAIRS * 2 * W], f32, tag="ws")

                for pp in range(GROUP_PAIRS):
                    off = pp * PAIR
                    g_ps = main_pool.tile([C, PAIR], f32, tag="pre", name="g_ps")
                    m_ps = main_pool.tile([C, PAIR], f32, tag="pre", name="m_ps")
                    # inject pre-added row/col projections (ident loaded once),
                    # then accumulate the edge_attr projection (+bias row)
                    for h in (0, 1):
                        hs = slice(off + h * TILE, off + (h + 1) * TILE)
                        ps = slice(h * TILE, (h + 1) * TILE)
                        nc.tensor.matmul(g_ps[:, ps], ident[:], preG_t[:, hs],
                                         start=True, stop=False, skip_group_check=True)
                        nc.tensor.matmul(m_ps[:, ps], ident[:], preM_t[:, hs],
                                         start=True, stop=False, skip_group_check=True)
                    for h in (0, 1):
                        hs = slice(off + h * TILE, off + (h + 1) * TILE)
                        ps = slice(h * TILE, (h + 1) * TILE)
                        nc.tensor.matmul(g_ps[:, ps], w3g[:], attr_t[:, hs],
                                         start=False, stop=True, skip_group_check=True)
                    for h in (0, 1):
                        hs = slice(off + h * TILE, off + (h + 1) * TILE)
                        ps = slice(h * TILE, (h + 1) * TILE)
                        nc.tensor.matmul(m_ps[:, ps], w3m[:], attr_t[:, hs],
                                         start=False, stop=True, skip_group_check=True)

                    # t1 = exp(-g), t2 = exp(c), sp = ln(t2 + 1)
                    t1 = epool.tile([C, PAIR], f32, tag="t1")
                    nc.scalar.activation(t1[:], g_ps[:], AF.Exp)
                    t2 = epool.tile([C, PAIR], bf, tag="t2")
                    nc.scalar.activation(t2[:], m_ps[:], AF.Exp)
                    sp = epool.tile([C, PAIR], bf, tag="sp")
                    nc.scalar.activation(sp[:], t2[:], AF.Ln, bias=1.0)

                    # m = sp / (1 + t1); the +1 runs on the otherwise-idle
                    # gpsimd engine, the reciprocal needs fp32 (bit-trick)
                    wd = epool.tile([C, PAIR], f32, tag="wd")
                    nc.gpsimd.tensor_scalar_add(wd[:], t1[:], 1.0)
                    rc = epool.tile([C, PAIR], f32, tag="rc")
                    nc.vector.reciprocal_approx_fast(rc[:], wd[:])
                    m = epool.tile([C, PAIR], bf, tag="m")
                    nc.vector.tensor_tensor(m[:], sp[:], rc[:], AluOpType.mult)

                    # transpose m -> [edges, ch] on the PE
                    mt_ps = t_pool.tile([128, PAIR], bf, tag="mt")
                    for k in range(8):
                        ks = slice(k * 128, (k + 1) * 128)
                        nc.tensor.transpose(mt_ps[:, ks], m[:, ks], ident[:])
                    m_t = epool.tile([128, PAIR], bf, tag="m_t")
                    nc.vector.tensor_copy(m_t[:], mt_ps[:])

                    # segment windows: host-built one-hot B, one window per
                    # 512-edge tile
                    for h in (0, 1):
                        seg_ps = s_pool.tile([C, W], f32, tag="seg", name="seg_ps")
                        for j in range(4):
                            k = h * 4 + j
                            cidx = (pp * 8 + k) * W
                            nc.tensor.matmul(seg_ps[:], m_t[:, k * 128:(k + 1) * 128],
                                             b_t[:, cidx:cidx + W],
                                             start=(j == 0), stop=(j == 3),
                                             skip_group_check=True)
                        wsl = slice((pp * 2 + h) * W, (pp * 2 + h + 1) * W)
                        wsum = wstage[:, wsl]
                        nc.vector.tensor_copy(wsum, seg_ps[:])

                nc.sync.dma_start(
                    wsums_d[:, g * GROUP_PAIRS * 2 * W:(g + 1) * GROUP_PAIRS * 2 * W],
                    wstage[:])

    # Force every activation onto natural_log_exp_and_others (exp AND ln):
    # the stock chooser greedily alternates exp_and_others/natural_log,
    # inserting ~2 table loads (~2.6us) per tile.
    import concourse.bacc as _bacc
    real_get = _bacc.get_activation_tables

    def pinned_tables(arch):
        tabs = real_get(arch)
        return {name: (funcs if name == "natural_log_exp_and_others" else set())
                for name, funcs in tabs.items()}

    _bacc.get_activation_tables = pinned_tables
    try:
        nc.compile()
    finally:
        _bacc.get_activation_tables = real_get
    return nc


def _postprocess(x, results, merge_info, meta):
    n_sup = meta["n_sup"]
    W = meta["w_seg"]
    out = np.empty((N_NODES, C), dtype=np.float32)
    for i in range(N_CORES):
        wsums = np.asarray(results[i]["wsums"], np.float32).reshape(C, n_sup, W)
        agg = np.zeros((C, SHARD_PAD + W), dtype=np.float32)
        bases, _ = merge_info[i]
        for t in range(n_sup):
            b = bases[t]
            agg[:, b:b + W] += wsums[:, t, :]
        out[i * SHARD:(i + 1) * SHARD] = agg[:, :SHARD].T
    out += np.asarray(x, np.float32)
    return out


_CACHE = {}


def kernel(**inputs):
    from concourse.bass_utils import run_bass_kernel_spmd

    in_maps, meta, merge_info = _prep(**inputs)
    key = (meta["e_pad"],)
    if key not in _CACHE:
        _CACHE[key] = _build(meta)
    nc = _CACHE[key]
    res = run_bass_kernel_spmd(nc, in_maps, core_ids=list(range(N_CORES)))
    return _postprocess(inputs["x"], res.results, merge_info, meta)


# revision 4
# speedup vs baseline: 3.9805x; 3.9805x over previous
"""CGConv message-passing kernel for 8 Trainium2 NeuronCores.

Strategy (self-contained; shapes hardcoded for the nn_CGConv problem):
 - Sort edges by destination node (col); shard edges into 8 buckets by
   col-range so every core owns a disjoint slice of output nodes (no
   collective needed).
 - Host precomputes per-node projections xg1 = x@gw1.T, xg2 = x@gw2.T
   (and msg twins) and ships per-edge pre-added tables
   G = -(xg1[row]+xg2[col]) and M = xm1[row]+xm2[col] as channel-major
   bf16 so the device streams them with contiguous DMA.  The gate side
   is negated so exp(gate_psum) == exp(-g) without an extra scale.
 - Device, per 1024-edge pair of tiles: inject G/M into PSUM via an
   identity matmul, accumulate the edge_attr projection (bias folded in
   via a ones-row), then t1=exp(-g), t2=exp(c), sp=ln(1+t2) on the ACT
   engine (natural_log_exp table pinned); sigma = 1/(1+t1) via the
   fast-reciprocal custom DVE op (bf16 out) and m = sp*sigma on DVE
   (uniform dtypes only - mixed-dtype tensor_tensor is ~10x slower).
 - Messages are PE-transposed to [edge, ch] and segment-summed by
   destination via host-built one-hot matmuls into a 64-node window per
   512-edge tile.  The transposes/segment matmuls of pair p are emitted
   inside pair p+1's iteration (software pipeline) so the PE never
   blocks on the DVE chain.
 - All DMA is issued per 5-pair group (5120 edges) to keep the Sync
   engine's ~600ns descriptor-issue cost off the critical path.
 - The host merges windows, adds the residual, and assembles the full
   [25000, 128] output.
"""

import numpy as np
import ml_dtypes

BF16 = ml_dtypes.bfloat16

N_NODES = 25000
N_EDGES = 400000
C = 128
EC = 64
N_CORES = 8
SHARD = 3125          # nodes per core
SHARD_PAD = 3200      # padded (multiple of 128)
TILE = 512            # edges per tile (one segment window each)
PAIR = 1024           # edges per compute step
GROUP_PAIRS = 5       # pairs per DMA group


def _prep(x, edge_index, edge_attr, gate_w, gate_b, msg_w, msg_b):
    row = np.asarray(edge_index[0], dtype=np.int64)
    col = np.asarray(edge_index[1], dtype=np.int64)
    x = np.asarray(x, dtype=np.float32)
    edge_attr = np.asarray(edge_attr, dtype=np.float32)
    gw = np.asarray(gate_w, np.float32)
    mw = np.asarray(msg_w, np.float32)
    gb = np.asarray(gate_b, np.float32)
    mb = np.asarray(msg_b, np.float32)

    order = np.argsort(col, kind="stable")
    row_s, col_s = row[order], col[order]
    attr_s = edge_attr[order]

    shard_of = col_s // SHARD
    starts = np.searchsorted(shard_of, np.arange(N_CORES))
    ends = np.searchsorted(shard_of, np.arange(N_CORES), side="right")
    sizes = ends - starts
    estep = PAIR * GROUP_PAIRS
    e_pad = int(-(-sizes.max() // estep) * estep)
    n_pairs = e_pad // PAIR
    n_sup = e_pad // TILE

    # per-node projections (f32 on host; quantized to bf16 after pre-add)
    xg1 = x @ gw[:, 0:C].T
    xg2 = x @ gw[:, C:2 * C].T
    xm1 = x @ mw[:, 0:C].T
    xm2 = x @ mw[:, C:2 * C].T

    # segment-window width: covers the max col-span of any 512-edge tile
    max_span = 0
    for i in range(N_CORES):
        c_l = col_s[starts[i]:ends[i]] - i * SHARD
        for t0 in range(0, len(c_l), TILE):
            seg = c_l[t0:t0 + TILE]
            max_span = max(max_span, int(seg[-1]) - int(seg[0]))
    Wseg = int(-(-(max_span + 3) // 32) * 32)  # pad-edge slack + round to 32
    Wseg = max(64, min(Wseg, 192))

    in_maps = []
    merge_info = []
    for i in range(N_CORES):
        sl = slice(starts[i], ends[i])
        r_i = row_s[sl]
        c_loc = (col_s[sl] - i * SHARD).astype(np.int64)
        a_i = attr_s[sl]
        n_i = sizes[i]
        pad = e_pad - n_i

        # pad col: must be >= SHARD (dropped at merge) and within W of its
        # tile's base. Real cols end <= 3124; pads go to a dedicated node id.
        if pad:
            last_real = int(c_loc[-1]) if n_i else 0
            pad_col = max(SHARD, last_real + 1)
            assert pad_col < SHARD_PAD
            r_i = np.concatenate([r_i, np.zeros(pad, np.int64)])
            c_loc = np.concatenate([c_loc, np.full(pad, pad_col, np.int64)])
            a_i = np.concatenate([a_i, np.zeros((pad, EC), np.float32)])

        bases = []
        for t in range(n_sup):
            seg = c_loc[t * TILE:(t + 1) * TILE]
            b = int(seg[0])
            assert int(seg[-1]) - b < Wseg, (
                f"core {i} tile {t}: col span {int(seg[-1]) - b} >= W={Wseg}")
            bases.append(b)
        bases_arr = np.repeat(np.asarray(bases, np.int64), TILE)

        c_glob = np.minimum(c_loc + i * SHARD, N_NODES - 1)
        # pre-added row+col projections, channel-major; gate side negated so
        # the device's exp(gate_psum) directly yields exp(-g)
        G = np.ascontiguousarray(-(xg1[r_i] + xg2[c_glob]).T).astype(BF16)
        M = np.ascontiguousarray((xm1[r_i] + xm2[c_glob]).T).astype(BF16)

        attrT = np.empty((EC + 1, e_pad), dtype=BF16)
        attrT[:EC] = a_i.T
        attrT[EC] = 1.0

        # one-hot segment matrices: B[e, w] = (col[e] - base == w), stored
        # [128 (edge-in-chunk), n_chunks * W] so chunk c sits at cols c*W
        colsub = (c_loc - bases_arr).astype(np.int64)
        assert colsub.min() >= 0 and colsub.max() < Wseg
        bmat = np.zeros((e_pad, Wseg), dtype=BF16)
        bmat[np.arange(e_pad), colsub] = 1.0
        bmat = np.ascontiguousarray(
            bmat.reshape(e_pad // 128, 128, Wseg).transpose(1, 0, 2)
        ).reshape(128, -1)

        in_maps.append({"preG": G, "preM": M, "attrT": attrT, "bmat": bmat})
        merge_info.append((bases, n_i))

    # shared (replicated) weight tables; gate side negated (bias folded)
    w3g = np.empty((EC + 1, C), dtype=BF16)
    w3g[:EC] = -gw[:, 2 * C:].T
    w3g[EC] = -gb
    w3m = np.empty((EC + 1, C), dtype=BF16)
    w3m[:EC] = mw[:, 2 * C:].T
    w3m[EC] = mb
    ident = np.eye(128, dtype=BF16)
    shared = {"w3g": w3g, "w3m": w3m, "ident": ident}
    for m in in_maps:
        m.update(shared)

    meta = {"e_pad": e_pad, "n_pairs": n_pairs, "n_sup": n_sup, "w_seg": Wseg}
    return in_maps, meta, merge_info


def _build(meta):
    import concourse.bacc as bacc
    import concourse.mybir as mybir
    from concourse import tile
    from concourse.alu_op_type import AluOpType
    from concourse.dve_ops import RECIP_APPROX_FAST_CONSTS, RECIPROCAL_APPROX_FAST

    e_pad, n_pairs, W = meta["e_pad"], meta["n_pairs"], meta["w_seg"]
    assert n_pairs % GROUP_PAIRS == 0
    GE = GROUP_PAIRS * PAIR              # edges per group
    bf = mybir.dt.bfloat16
    f32 = mybir.dt.float32
    AF = mybir.ActivationFunctionType
    RC = RECIP_APPROX_FAST_CONSTS

    nc = bacc.Bacc(None, target_bir_lowering=False, debug=False)

    preG_d = nc.declare_dram_parameter("preG", [C, e_pad], bf, isOutput=False)
    preM_d = nc.declare_dram_parameter("preM", [C, e_pad], bf, isOutput=False)
    attr_d = nc.declare_dram_parameter("attrT", [EC + 1, e_pad], bf, isOutput=False)
    bmat_d = nc.declare_dram_parameter("bmat", [128, (e_pad // 128) * W], bf, isOutput=False)
    ident_d = nc.declare_dram_parameter("ident", [128, 128], bf, isOutput=False)
    w3g_d = nc.declare_dram_parameter("w3g", [EC + 1, C], bf, isOutput=False)
    w3m_d = nc.declare_dram_parameter("w3m", [EC + 1, C], bf, isOutput=False)
    wsums_d = nc.declare_dram_parameter("wsums", [C, (e_pad // TILE) * W], f32, isOutput=True)

    with tile.TileContext(nc) as tc:
        with (
            tc.tile_pool(name="const", bufs=1) as cpool,
            tc.tile_pool(name="grp", bufs=2) as gpool,
            tc.tile_pool(name="wout", bufs=2) as wpool,
            tc.tile_pool(name="elem", bufs=3) as epool,
            tc.tile_pool(name="mps", bufs=3, space="PSUM") as main_pool,
            tc.tile_pool(name="tps", bufs=1, space="PSUM") as t_pool,
            tc.tile_pool(name="sps", bufs=1, space="PSUM") as s_pool,
        ):
            ident = cpool.tile([128, 128], bf, tag="ident")
            nc.sync.dma_start(ident[:], ident_d[:])
            w3g = cpool.tile([EC + 1, C], bf, tag="w3g")
            nc.sync.dma_start(w3g[:], w3g_d[:])
            w3m = cpool.tile([EC + 1, C], bf, tag="w3m")
            nc.sync.dma_start(w3m[:], w3m_d[:])

            # ~8us of dense back-to-back matmuls: one continuous burst longer
            # than the HAM activity window so the PE clock unthrottles to
            # 2.4GHz before the real stream.
            warm_in = cpool.tile([128, TILE], bf, tag="warm")
            nc.gpsimd.memset(warm_in[:], 0.0)
            warm_ps = s_pool.tile([C, 2 * W], f32, tag="seg", name="warm_ps")
            for _ in range(90):
                nc.tensor.matmul(warm_ps[:, 0:W], warm_in[:, 0:128], warm_in[:, 0:W],
                                 start=True, stop=True, skip_group_check=True)

            # software pipeline: the transpose + segment matmuls of pair p are
            # emitted during pair p+1, after its main matmuls, so the PE is
            # never queued behind the DVE chain of the same pair.
            pend = None   # (m, b_t, wstage, pp)
            grp_state = {}

            def emit_shorts(state):
                m, b_t, wstage, pp = state
                mt_ps = t_pool.tile([128, PAIR], bf, tag="mt", name="mt_ps")
                for k in range(8):
                    ks = slice(k * 128, (k + 1) * 128)
                    nc.tensor.transpose(mt_ps[:, ks], m[:, ks], ident[:])
                m_t = epool.tile([128, PAIR], bf, tag="m_t", name="m_t")
                nc.vector.tensor_copy(m_t[:], mt_ps[:])
                seg_ps = s_pool.tile([C, 2 * W], f32, tag="seg", name="seg_ps")
                for k in range(8):
                    h = k // 4
                    cidx = (pp * 8 + k) * W
                    nc.tensor.matmul(seg_ps[:, h * W:(h + 1) * W],
                                     m_t[:, k * 128:(k + 1) * 128],
                                     b_t[:, cidx:cidx + W],
                                     start=(k % 4 == 0), stop=(k % 4 == 3),
                                     skip_group_check=True)
                wsum = wstage[:, pp * 2 * W:(pp + 1) * 2 * W]
                nc.vector.tensor_copy(wsum, seg_ps[:])

            for p in range(n_pairs):
                pp = p % GROUP_PAIRS
                if pp == 0:
                    g = p // GROUP_PAIRS
                    gsl = slice(g * GE, (g + 1) * GE)
                    preG_t = gpool.tile([C, GE], bf, tag="pg", name="preG_t")
                    nc.sync.dma_start(preG_t[:], preG_d[:, gsl])
                    preM_t = gpool.tile([C, GE], bf, tag="pm", name="preM_t")
                    nc.sync.dma_start(preM_t[:], preM_d[:, gsl])
                    attr_t = gpool.tile([EC + 1, GE], bf, tag="at", name="attr_t")
                    nc.sync.dma_start(attr_t[:], attr_d[:, gsl])
                    b_t = gpool.tile([128, GROUP_PAIRS * 8 * W], bf, tag="bt", name="b_t")
                    nc.sync.dma_start(b_t[:], bmat_d[:, g * GROUP_PAIRS * 8 * W:
                                                     (g + 1) * GROUP_PAIRS * 8 * W])
                    wstage = wpool.tile([C, GROUP_PAIRS * 2 * W], f32, tag="ws",
                                        name="wstage")
                    grp_state = {"preG": preG_t, "preM": preM_t, "attr": attr_t,
                                 "b": b_t, "ws": wstage, "g": g}

                off = pp * PAIR
                g_ps = main_pool.tile([C, PAIR], f32, tag="pre", name="g_ps")
                m_ps = main_pool.tile([C, PAIR], f32, tag="pre", name="m_ps")
                for h in (0, 1):
                    hs = slice(off + h * TILE, off + (h + 1) * TILE)
                    ps = slice(h * TILE, (h + 1) * TILE)
                    nc.tensor.matmul(g_ps[:, ps], ident[:], grp_state["preG"][:, hs],
                                     start=True, stop=False, skip_group_check=True)
                    nc.tensor.matmul(m_ps[:, ps], ident[:], grp_state["preM"][:, hs],
                                     start=True, stop=False, skip_group_check=True)
                for h in (0, 1):
                    hs = slice(off + h * TILE, off + (h + 1) * TILE)
                    ps = slice(h * TILE, (h + 1) * TILE)
                    nc.tensor.matmul(g_ps[:, ps], w3g[:], grp_state["attr"][:, hs],
                                     start=False, stop=True, skip_group_check=True)
                for h in (0, 1):
                    hs = slice(off + h * TILE, off + (h + 1) * TILE)
                    ps = slice(h * TILE, (h + 1) * TILE)
                    nc.tensor.matmul(m_ps[:, ps], w3m[:], grp_state["attr"][:, hs],
                                     start=False, stop=True, skip_group_check=True)

                if pend is not None:
                    emit_shorts(pend)
                    if pp == 0 and p > 0:
                        pg = p // GROUP_PAIRS - 1
                        nc.sync.dma_start(
                            wsums_d[:, pg * GROUP_PAIRS * 2 * W:
                                    (pg + 1) * GROUP_PAIRS * 2 * W],
                            pend[2][:])

                # t1 = exp(-g), t2 = exp(c), sp = ln(t2 + 1)
                t1 = epool.tile([C, PAIR], f32, tag="t1")
                nc.scalar.activation(t1[:], g_ps[:], AF.Exp)
                t2 = epool.tile([C, PAIR], bf, tag="t2")
                nc.scalar.activation(t2[:], m_ps[:], AF.Exp)
                sp = epool.tile([C, PAIR], bf, tag="sp")
                nc.scalar.activation(sp[:], t2[:], AF.Ln, bias=1.0)

                # sigma = 1/(1 + t1) (bf16 out), m = sp * sigma
                wd = epool.tile([C, PAIR], f32, tag="wd")
                nc.vector.tensor_scalar_add(wd[:], t1[:], 1.0)
                rc = epool.tile([C, PAIR], bf, tag="rc")
                nc.vector._custom_dve(RECIPROCAL_APPROX_FAST, out=rc[:], in0=wd[:],
                                      s0=RC["s0"], s1=RC["s1"], imm2=RC["imm2"])
                m = epool.tile([C, PAIR], bf, tag="m")
                nc.vector.tensor_tensor(m[:], sp[:], rc[:], AluOpType.mult)

                pend = (m, grp_state["b"], grp_state["ws"], pp)

            emit_shorts(pend)
            pg = n_pairs // GROUP_PAIRS - 1
            nc.sync.dma_start(
                wsums_d[:, pg * GROUP_PAIRS * 2 * W:(pg + 1) * GROUP_PAIRS * 2 * W],
                pend[2][:])

    # Force every activation onto natural_log_exp_and_others (exp AND ln):
    # the stock chooser greedily alternates exp_and_others/natural_log,
    # inserting ~2 table loads (~2.6us) per tile.
    import concourse.bacc as _bacc
    real_get = _bacc.get_activation_tables

    def pinned_tables(arch):
        tabs = real_get(arch)
        return {name: (funcs if name == "natural_log_exp_and_others" else set())
                for name, funcs in tabs.items()}

    _bacc.get_activation_tables = pinned_tables
    try:
        nc.compile()
    finally:
        _bacc.get_activation_tables = real_get
    return nc


def _postprocess(x, results, merge_info, meta):
    n_sup = meta["n_sup"]
    W = meta["w_seg"]
    out = np.empty((N_NODES, C), dtype=np.float32)
    for i in range(N_CORES):
        wsums = np.asarray(results[i]["wsums"], np.float32).reshape(C, n_sup, W)
        agg = np.zeros((C, SHARD_PAD + W), dtype=np.float32)
        bases, _ = merge_info[i]
        for t in range(n_sup):
            b = bases[t]
            agg[:, b:b + W] += wsums[:, t, :]
        out[i * SHARD:(i + 1) * SHARD] = agg[:, :SHARD].T
    out += np.asarray(x, np.float32)
    return out


_CACHE = {}


def kernel(**inputs):
    from concourse.bass_utils import run_bass_kernel_spmd

    in_maps, meta, merge_info = _prep(**inputs)
    key = (meta["e_pad"],)
    if key not in _CACHE:
        _CACHE[key] = _build(meta)
    nc = _CACHE[key]
    res = run_bass_kernel_spmd(nc, in_maps, core_ids=list(range(N_CORES)))
    return _postprocess(inputs["x"], res.results, merge_info, meta)


# revision 10
# speedup vs baseline: 4.1363x; 1.0391x over previous
"""CGConv message-passing kernel for 8 Trainium2 NeuronCores.

Strategy (self-contained; shapes hardcoded for the nn_CGConv problem):
 - Sort edges by destination node (col); shard edges into 8 buckets by
   col-range so every core owns a disjoint slice of output nodes (no
   collective needed).
 - Host precomputes per-node projections xg1 = x@gw1.T, xg2 = x@gw2.T
   (and msg twins) and ships per-edge pre-added tables
   G = -(xg1[row]+xg2[col]) and M = xm1[row]+xm2[col] as channel-major
   bf16 so the device streams them with contiguous DMA.  The gate side
   is negated so exp(gate_psum) == exp(-g) without an extra scale.
 - Device, per 1024-edge pair of tiles: inject G/M into PSUM via an
   identity matmul, accumulate the edge_attr projection (bias folded in
   via a ones-row), then t1=exp(-g), t2=exp(c), sp=ln(1+t2) on the ACT
   engine (natural_log_exp table pinned); sigma = 1/(1+t1) via the
   fast-reciprocal custom DVE op (bf16 out) and m = sp*sigma on DVE
   (uniform dtypes only - mixed-dtype tensor_tensor is ~10x slower).
 - Messages are PE-transposed to [edge, ch] and segment-summed by
   destination via host-built one-hot matmuls into a 64-node window per
   512-edge tile.  The transposes/segment matmuls of pair p are emitted
   inside pair p+1's iteration (software pipeline) so the PE never
   blocks on the DVE chain.
 - All DMA is issued per 5-pair group (5120 edges) to keep the Sync
   engine's ~600ns descriptor-issue cost off the critical path.
 - The host merges windows, adds the residual, and assembles the full
   [25000, 128] output.
"""

import numpy as np
import ml_dtypes

BF16 = ml_dtypes.bfloat16

N_NODES = 25000
N_EDGES = 400000
C = 128
EC = 64
N_CORES = 8
SHARD = 3125          # nodes per core
SHARD_PAD = 3200      # padded (multiple of 128)
TILE = 512            # edges per tile (one segment window each)
PAIR = 1024           # edges per compute step
GROUP_PAIRS = 5       # pairs per DMA group


def _prep(x, edge_index, edge_attr, gate_w, gate_b, msg_w, msg_b):
    row = np.asarray(edge_index[0], dtype=np.int64)
    col = np.asarray(edge_index[1], dtype=np.int64)
    x = np.asarray(x, dtype=np.float32)
    edge_attr = np.asarray(edge_attr, dtype=np.float32)
    gw = np.asarray(gate_w, np.float32)
    mw = np.asarray(msg_w, np.float32)
    gb = np.asarray(gate_b, np.float32)
    mb = np.asarray(msg_b, np.float32)

    order = np.argsort(col, kind="stable")
    row_s, col_s = row[order], col[order]
    attr_s = edge_attr[order]

    shard_of = col_s // SHARD
    starts = np.searchsorted(shard_of, np.arange(N_CORES))
    ends = np.searchsorted(shard_of, np.arange(N_CORES), side="right")
    sizes = ends - starts
    estep = PAIR * GROUP_PAIRS
    e_pad = int(-(-sizes.max() // estep) * estep)
    n_pairs = e_pad // PAIR
    n_sup = e_pad // TILE

    # per-node projections (f32 on host; quantized to bf16 after pre-add)
    xg1 = x @ gw[:, 0:C].T
    xg2 = x @ gw[:, C:2 * C].T
    xm1 = x @ mw[:, 0:C].T
    xm2 = x @ mw[:, C:2 * C].T

    # segment-window width: covers the max col-span of any 512-edge tile
    max_span = 0
    for i in range(N_CORES):
        c_l = col_s[starts[i]:ends[i]] - i * SHARD
        for t0 in range(0, len(c_l), TILE):
            seg = c_l[t0:t0 + TILE]
            max_span = max(max_span, int(seg[-1]) - int(seg[0]))
    Wseg = int(-(-(max_span + 3) // 32) * 32)  # pad-edge slack + round to 32
    Wseg = max(64, min(Wseg, 192))

    in_maps = []
    merge_info = []
    for i in range(N_CORES):
        sl = slice(starts[i], ends[i])
        r_i = row_s[sl]
        c_loc = (col_s[sl] - i * SHARD).astype(np.int64)
        a_i = attr_s[sl]
        n_i = sizes[i]
        pad = e_pad - n_i

        # pad col: must be >= SHARD (dropped at merge) and within W of its
        # tile's base. Real cols end <= 3124; pads go to a dedicated node id.
        if pad:
            last_real = int(c_loc[-1]) if n_i else 0
            pad_col = max(SHARD, last_real + 1)
            assert pad_col < SHARD_PAD
            r_i = np.concatenate([r_i, np.zeros(pad, np.int64)])
            c_loc = np.concatenate([c_loc, np.full(pad, pad_col, np.int64)])
            a_i = np.concatenate([a_i, np.zeros((pad, EC), np.float32)])

        bases = []
        for t in range(n_sup):
            seg = c_loc[t * TILE:(t + 1) * TILE]
            b = int(seg[0])
            assert int(seg[-1]) - b < Wseg, (
                f"core {i} tile {t}: col span {int(seg[-1]) - b} >= W={Wseg}")
            bases.append(b)
        bases_arr = np.repeat(np.asarray(bases, np.int64), TILE)

        c_glob = np.minimum(c_loc + i * SHARD, N_NODES - 1)
        # pre-added row+col projections, channel-major; gate side negated so
        # the device's exp(gate_psum) directly yields exp(-g)
        G = np.ascontiguousarray(-(xg1[r_i] + xg2[c_glob]).T).astype(BF16)
        M = np.ascontiguousarray((xm1[r_i] + xm2[c_glob]).T).astype(BF16)

        attrT = np.empty((EC + 1, e_pad), dtype=BF16)
        attrT[:EC] = a_i.T
        attrT[EC] = 1.0

        # one-hot segment matrices: B[e, w] = (col[e] - base == w), stored
        # [128 (edge-in-chunk), n_chunks * W] so chunk c sits at cols c*W
        colsub = (c_loc - bases_arr).astype(np.int64)
        assert colsub.min() >= 0 and colsub.max() < Wseg
        bmat = np.zeros((e_pad, Wseg), dtype=BF16)
        bmat[np.arange(e_pad), colsub] = 1.0
        bmat = np.ascontiguousarray(
            bmat.reshape(e_pad // 128, 128, Wseg).transpose(1, 0, 2)
        ).reshape(128, -1)

        in_maps.append({"preG": G, "preM": M, "attrT": attrT, "bmat": bmat})
        merge_info.append((bases, n_i))

    # shared (replicated) weight tables; gate side negated (bias folded)
    w3g = np.empty((EC + 1, C), dtype=BF16)
    w3g[:EC] = -gw[:, 2 * C:].T
    w3g[EC] = -gb
    w3m = np.empty((EC + 1, C), dtype=BF16)
    w3m[:EC] = mw[:, 2 * C:].T
    w3m[EC] = mb
    ident = np.eye(128, dtype=BF16)
    shared = {"w3g": w3g, "w3m": w3m, "ident": ident}
    for m in in_maps:
        m.update(shared)

    meta = {"e_pad": e_pad, "n_pairs": n_pairs, "n_sup": n_sup, "w_seg": Wseg}
    return in_maps, meta, merge_info


def _register_recip1p():
    """Custom single-pass DVE op: out = 1/(1+x) via the BITWISE_NOT
    exponent-flip seed + one Newton step (~0.17% max rel err).  Same
    Chebyshev constants as RECIPROCAL_APPROX_FAST (they are minimax-optimal
    for the single-NR truncation); the +1 rides the spare C2 slot."""
    import numpy as np
    from concourse import dve_ops as D
    from concourse.dve_spec import AluOp, Bin, C0, C1, C2, Spec, Src0

    for op in D.OPS:
        if op.name == "RECIP1P_ANT":
            return op

    x1 = Src0 + C2
    nx = Bin(AluOp.BITWISE_NOT, x1, x1)
    y0 = nx * C0
    body = y0 * (C1 - x1 * y0)

    def ref(in0, in1, c0, c1, c2):
        xx = (in0.astype(np.float32) + np.float32(c2)).astype(np.float32)
        nxx = (~xx.view(np.int32)).view(np.float32)
        yy0 = nxx * np.float32(c0)
        return yy0 * (np.float32(c1) - xx * yy0)

    spec = Spec(body=body, reference=ref)
    op = D.DveOp(
        "RECIP1P_ANT", spec, subdim=False,
        uops_sha={"v3": "aa55afded45a0392", "v4": "7cf22af25044d172"})
    D._SUB_OPCODE_FOR_NAME[op.name] = max(D._SUB_OPCODE_FOR_NAME.values()) + 1
    D.OPS.append(op)
    D.CUSTOM_DVE_SPECS[op.name] = spec
    return op


def _build(meta):
    import concourse.bacc as bacc
    import concourse.mybir as mybir
    from concourse import tile
    from concourse.alu_op_type import AluOpType

    recip1p = _register_recip1p()

    e_pad, n_pairs, W = meta["e_pad"], meta["n_pairs"], meta["w_seg"]
    assert n_pairs % GROUP_PAIRS == 0
    GE = GROUP_PAIRS * PAIR              # edges per group
    bf = mybir.dt.bfloat16
    f32 = mybir.dt.float32
    AF = mybir.ActivationFunctionType

    nc = bacc.Bacc(None, target_bir_lowering=False, debug=False)

    preG_d = nc.declare_dram_parameter("preG", [C, e_pad], bf, isOutput=False)
    preM_d = nc.declare_dram_parameter("preM", [C, e_pad], bf, isOutput=False)
    attr_d = nc.declare_dram_parameter("attrT", [EC + 1, e_pad], bf, isOutput=False)
    bmat_d = nc.declare_dram_parameter("bmat", [128, (e_pad // 128) * W], bf, isOutput=False)
    ident_d = nc.declare_dram_parameter("ident", [128, 128], bf, isOutput=False)
    w3g_d = nc.declare_dram_parameter("w3g", [EC + 1, C], bf, isOutput=False)
    w3m_d = nc.declare_dram_parameter("w3m", [EC + 1, C], bf, isOutput=False)
    wsums_d = nc.declare_dram_parameter("wsums", [C, (e_pad // TILE) * W], f32, isOutput=True)

    with tile.TileContext(nc) as tc:
        with (
            tc.tile_pool(name="const", bufs=1) as cpool,
            tc.tile_pool(name="grp", bufs=2) as gpool,
            tc.tile_pool(name="wout", bufs=2) as wpool,
            tc.tile_pool(name="elem", bufs=3) as epool,
            tc.tile_pool(name="mps", bufs=3, space="PSUM") as main_pool,
            tc.tile_pool(name="tps", bufs=1, space="PSUM") as t_pool,
            tc.tile_pool(name="sps", bufs=1, space="PSUM") as s_pool,
        ):
            def issue_group(g):
                gsl = slice(g * GE, (g + 1) * GE)
                preG_t = gpool.tile([C, GE], bf, tag="pg", name="preG_t")
                nc.sync.dma_start(preG_t[:], preG_d[:, gsl])
                preM_t = gpool.tile([C, GE], bf, tag="pm", name="preM_t")
                nc.sync.dma_start(preM_t[:], preM_d[:, gsl])
                attr_t = gpool.tile([EC + 1, GE], bf, tag="at", name="attr_t")
                nc.sync.dma_start(attr_t[:], attr_d[:, gsl])
                b_t = gpool.tile([128, GROUP_PAIRS * 8 * W], bf, tag="bt", name="b_t")
                nc.sync.dma_start(b_t[:], bmat_d[:, g * GROUP_PAIRS * 8 * W:
                                                 (g + 1) * GROUP_PAIRS * 8 * W])
                wstage = wpool.tile([C, GROUP_PAIRS * 2 * W], f32, tag="ws",
                                    name="wstage")
                return {"preG": preG_t, "preM": preM_t, "attr": attr_t,
                        "b": b_t, "ws": wstage, "g": g}

            # group 0 input DMAs go out first so the transfers overlap the
            # PE warm-up burst
            grp0 = issue_group(0)
            ident = cpool.tile([128, 128], bf, tag="ident")
            nc.sync.dma_start(ident[:], ident_d[:])
            w3g = cpool.tile([EC + 1, C], bf, tag="w3g")
            nc.sync.dma_start(w3g[:], w3g_d[:])
            w3m = cpool.tile([EC + 1, C], bf, tag="w3m")
            nc.sync.dma_start(w3m[:], w3m_d[:])

            # ~4us of dense back-to-back matmuls: one continuous burst longer
            # than the HAM activity window so the PE clock unthrottles before
            # the real stream (which never idles long enough to re-throttle).
            warm_in = cpool.tile([128, TILE], bf, tag="warm")
            nc.gpsimd.memset(warm_in[:], 0.0)
            warm_ps = s_pool.tile([C, 2 * W], f32, tag="seg", name="warm_ps")
            for _ in range(40):
                nc.tensor.matmul(warm_ps[:, 0:W], warm_in[:, 0:128], warm_in[:, 0:W],
                                 start=True, stop=True, skip_group_check=True)

            # software pipeline: the transpose + segment matmuls of pair p are
            # emitted during pair p+1 -- the 8 transposes interleaved between
            # its main matmuls (their LDWEIGHTS hide under the 512-col main
            # streams), the segment matmuls after -- so the PE is never queued
            # behind the DVE chain of the same pair.
            pend = None   # (m, b_t, wstage, pp)
            grp_state = {}

            def make_transposes(state):
                m, b_t, wstage, pp = state
                mt_ps = t_pool.tile([128, PAIR], bf, tag="mt", name="mt_ps")

                def one(k):
                    ks = slice(k * 128, (k + 1) * 128)
                    nc.tensor.transpose(mt_ps[:, ks], m[:, ks], ident[:])
                return mt_ps, one

            def emit_tail(state, mt_ps):
                m, b_t, wstage, pp = state
                m_t = epool.tile([128, PAIR], bf, tag="m_t", name="m_t")
                nc.vector.tensor_copy(m_t[:], mt_ps[:])
                seg_ps = s_pool.tile([C, 2 * W], f32, tag="seg", name="seg_ps")
                for k in range(8):
                    h = k // 4
                    cidx = (pp * 8 + k) * W
                    nc.tensor.matmul(seg_ps[:, h * W:(h + 1) * W],
                                     m_t[:, k * 128:(k + 1) * 128],
                                     b_t[:, cidx:cidx + W],
                                     start=(k % 4 == 0), stop=(k % 4 == 3),
                                     skip_group_check=True)
                wsum = wstage[:, pp * 2 * W:(pp + 1) * 2 * W]
                nc.vector.tensor_copy(wsum, seg_ps[:])

            for p in range(n_pairs):
                pp = p % GROUP_PAIRS
                if pp == 0:
                    g = p // GROUP_PAIRS
                    grp_state = grp0 if g == 0 else issue_group(g)

                off = pp * PAIR
                g_ps = main_pool.tile([C, PAIR], f32, tag="pre", name="g_ps")
                m_ps = main_pool.tile([C, PAIR], f32, tag="pre", name="m_ps")
                mains = []
                for h in (0, 1):
                    hs = slice(off + h * TILE, off + (h + 1) * TILE)
                    ps = slice(h * TILE, (h + 1) * TILE)
                    mains.append(lambda hs=hs, ps=ps: nc.tensor.matmul(
                        g_ps[:, ps], ident[:], grp_state["preG"][:, hs],
                        start=True, stop=False, skip_group_check=True))
                    mains.append(lambda hs=hs, ps=ps: nc.tensor.matmul(
                        m_ps[:, ps], ident[:], grp_state["preM"][:, hs],
                        start=True, stop=False, skip_group_check=True))
                for h in (0, 1):
                    hs = slice(off + h * TILE, off + (h + 1) * TILE)
                    ps = slice(h * TILE, (h + 1) * TILE)
                    mains.append(lambda hs=hs, ps=ps: nc.tensor.matmul(
                        g_ps[:, ps], w3g[:], grp_state["attr"][:, hs],
                        start=False, stop=True, skip_group_check=True))
                for h in (0, 1):
                    hs = slice(off + h * TILE, off + (h + 1) * TILE)
                    ps = slice(h * TILE, (h + 1) * TILE)
                    mains.append(lambda hs=hs, ps=ps: nc.tensor.matmul(
                        m_ps[:, ps], w3m[:], grp_state["attr"][:, hs],
                        start=False, stop=True, skip_group_check=True))

                if pend is not None:
                    mt_ps, one_t = make_transposes(pend)
                    for i, mm in enumerate(mains):
                        mm()
                        one_t(i)
                    for i in range(len(mains), 8):
                        one_t(i)
                    emit_tail(pend, mt_ps)
                    if pp == 0 and p > 0:
                        pg = p // GROUP_PAIRS - 1
                        nc.sync.dma_start(
                            wsums_d[:, pg * GROUP_PAIRS * 2 * W:
                                    (pg + 1) * GROUP_PAIRS * 2 * W],
                            pend[2][:])
                else:
                    for mm in mains:
                        mm()

                # t1 = exp(-g), t2 = exp(c), sp = ln(t2 + 1)
                t1 = epool.tile([C, PAIR], f32, tag="t1")
                nc.scalar.activation(t1[:], g_ps[:], AF.Exp)
                t2 = epool.tile([C, PAIR], bf, tag="t2")
                nc.scalar.activation(t2[:], m_ps[:], AF.Exp)
                sp = epool.tile([C, PAIR], bf, tag="sp")
                nc.scalar.activation(sp[:], t2[:], AF.Ln, bias=1.0)

                # sigma = 1/(1 + t1) in one fused DVE op (bf16 out), m = sp * sigma
                rc = epool.tile([C, PAIR], bf, tag="rc")
                nc.vector._custom_dve(recip1p, out=rc[:], in0=t1[:],
                                      s0=-0.23549792, s1=2.0017324, imm2=1.0)
                m = epool.tile([C, PAIR], bf, tag="m")
                nc.vector.tensor_tensor(m[:], sp[:], rc[:], AluOpType.mult)

                pend = (m, grp_state["b"], grp_state["ws"], pp)

            mt_ps, one_t = make_transposes(pend)
            for i in range(8):
                one_t(i)
            emit_tail(pend, mt_ps)
            pg = n_pairs // GROUP_PAIRS - 1
            nc.sync.dma_start(
                wsums_d[:, pg * GROUP_PAIRS * 2 * W:(pg + 1) * GROUP_PAIRS * 2 * W],
                pend[2][:])

    # Force every activation onto natural_log_exp_and_others (exp AND ln):
    # the stock chooser greedily alternates exp_and_others/natural_log,
    # inserting ~2 table loads (~2.6us) per tile.
    import concourse.bacc as _bacc
    real_get = _bacc.get_activation_tables

    def pinned_tables(arch):
        tabs = real_get(arch)
        return {name: (funcs if name == "natural_log_exp_and_others" else set())
                for name, funcs in tabs.items()}

    _bacc.get_activation_tables = pinned_tables
    try:
        nc.compile()
    finally:
        _bacc.get_activation_tables = real_get
    return nc


def _postprocess(x, results, merge_info, meta):
    n_sup = meta["n_sup"]
    W = meta["w_seg"]
    out = np.empty((N_NODES, C), dtype=np.float32)
    for i in range(N_CORES):
        wsums = np.asarray(results[i]["wsums"], np.float32).reshape(C, n_sup, W)
        agg = np.zeros((C, SHARD_PAD + W), dtype=np.float32)
        bases, _ = merge_info[i]
        for t in range(n_sup):
            b = bases[t]
            agg[:, b:b + W] += wsums[:, t, :]
        out[i * SHARD:(i + 1) * SHARD] = agg[:, :SHARD].T
    out += np.asarray(x, np.float32)
    return out


_CACHE = {}


def kernel(**inputs):
    from concourse.bass_utils import run_bass_kernel_spmd

    in_maps, meta, merge_info = _prep(**inputs)
    key = (meta["e_pad"],)
    if key not in _CACHE:
        _CACHE[key] = _build(meta)
    nc = _CACHE[key]
    res = run_bass_kernel_spmd(nc, in_maps, core_ids=list(range(N_CORES)))
    return _postprocess(inputs["x"], res.results, merge_info, meta)
